# revision 13
# baseline (speedup 1.0000x reference)
"""Trainium2 Bass kernel for nn_COMPASSWeightModel (GNN message passing).

Strategy:
- Sort edges by receiver node (host side); shard the sorted edge list across
  8 cores by receiver-node ranges (2048 nodes per core) so every scatter
  (segment softmax denominator + segment sum) is core-local.
- Edge tensors live feature-major ([256, E] tiles) so the MLP GEMMs stream
  edges as the moving operand with weights stationary; per-node one-hot
  matrices (built on-device from baked receiver indices) turn segment
  sum/expand into PE matmuls.
- The src-side node gather uses indirect DMA against an AllGathered node
  table P = r_v @ W_a (fp32, DRAM); the recv-side uses one-hot expand
  matmuls against the core-local Q = r_v @ W_b.
- softmax max-subtraction is dropped: scores are sigmoid outputs in (0,1),
  so exp(score)/segsum(exp(score)) is exact shift-invariant softmax.
- fp16 for GEMM operands/storage, fp32 PSUM accumulation and statistics.
- Final [64,768] graph head is assembled on host from device b_v (0.01% of
  FLOPs); w_e (the E-sized output) is computed on device.
"""
import numpy as np

N_NODES = 16384
N_EDGES = 262144
N_GRAPHS = 64
H = 256
NITERS = 3
NCORE = 8
NPC = N_NODES // NCORE          # nodes per core
NT = NPC // 128                 # node tiles per core (16)
P = 128

_CACHE = {}


def _split_multi_waits(nc, mybir, max_waits=1):
    """walrus here accepts one sync wait per instruction; split extras
    onto single-wait NoOps inserted before the offending instruction."""
    for func in nc.m.functions:
        for blk in func.blocks:
            out_insts = []
            changed = False
            for inst in blk.instructions:
                si = inst.sync_info
                if si is not None and si.on_wait and len(si.on_wait) > max_waits:
                    waits = list(si.on_wait)
                    keep = waits[-max_waits:]
                    for w in waits[:-max_waits]:
                        nop = mybir.InstNoOp(
                            name=nc.get_next_instruction_name(), ins=[], outs=[])
                        nop.engine = inst.engine
                        nop.sync_info = mybir.SyncInfo(on_wait=[w], on_update=[])
                        nc.register_instruction(nop)
                        out_insts.append(nop)
                    inst.sync_info = mybir.SyncInfo(
                        on_wait=keep, on_update=list(si.on_update))
                    changed = True
                out_insts.append(inst)
            if changed:
                blk.instructions[:] = out_insts


def _build_nc(EC, windows, last_tile):
    """Build the SPMD Bass program. EC = per-core edge capacity (mult of 512),
    windows[n] = (t_lo, t_hi) edge-tile window for node-tile n (uniform
    across cores), last_tile[n] = t_hi-1 per node tile."""
    import concourse.bass as bass
    import concourse.mybir as mybir
    import concourse.tile as tile

    f16 = mybir.dt.float16
    f32 = mybir.dt.float32
    i32 = mybir.dt.int32
    T = EC // P                 # edge tiles per core
    G = EC // 512               # edge groups (4 tiles)

    nc = bass.Bass(num_devices=NCORE)

    # ---- DRAM I/O ----
    d_be0 = nc.dram_tensor("be0", [2, P, EC], f16, kind="ExternalInput")
    d_recv = nc.dram_tensor("recvc", [P, T], f32, kind="ExternalInput")
    d_src = nc.dram_tensor("srcc", [P, T], i32, kind="ExternalInput")
    d_iota = nc.dram_tensor("iota", [P, P], f32, kind="ExternalInput")
    d_ndr = nc.dram_tensor("ndr", [P, NT], f32, kind="ExternalInput")
    d_invdeg = nc.dram_tensor("invdeg", [P, NT], f32, kind="ExternalInput")
    d_hhT = nc.dram_tensor("hhT", [2, NPC], f16, kind="ExternalInput")
    d_id16 = nc.dram_tensor("id16", [P, P], f16, kind="ExternalInput")
    d_id32 = nc.dram_tensor("id32", [P, P], f32, kind="ExternalInput")
    d_eaw1 = nc.dram_tensor("eaw1", [256, 256], f16, kind="ExternalInput")
    d_eab1 = nc.dram_tensor("eab1", [P, 2], f32, kind="ExternalInput")
    d_eaw2 = nc.dram_tensor("eaw2", [256, 1], f16, kind="ExternalInput")
    d_wa = nc.dram_tensor("wa", [256, 256], f16, kind="ExternalInput")
    d_wa2 = nc.dram_tensor("wa2", [2, 256], f16, kind="ExternalInput")
    d_wb = nc.dram_tensor("wb", [256, 256], f16, kind="ExternalInput")
    d_wb2 = nc.dram_tensor("wb2", [2, 256], f16, kind="ExternalInput")
    d_wc = nc.dram_tensor("wc", [256, 256], f16, kind="ExternalInput")
    d_eub1 = nc.dram_tensor("eub1", [P, 2], f32, kind="ExternalInput")
    d_euw2 = nc.dram_tensor("euw2", [256, 256], f16, kind="ExternalInput")
    d_eub2 = nc.dram_tensor("eub2", [P, 2], f32, kind="ExternalInput")
    d_eww = nc.dram_tensor("eww", [256, 1], f16, kind="ExternalInput")

    d_we = nc.dram_tensor("we_out", [P, T], f32, kind="ExternalOutput")
    d_bv = nc.dram_tensor("bv_out", [NPC, 256], f32, kind="ExternalOutput")

    d_beB = nc.dram_tensor("beB", [2, P, EC], f16, kind="Internal")
    d_beC = nc.dram_tensor("beC", [2, P, EC], f16, kind="Internal")
    be_seq = [d_be0, d_beB, d_beC]

    d_ccin = [nc.dram_tensor(f"ccin{i}", [NPC, 256], f32, kind="Internal")
              for i in range(NITERS)]
    d_ccP = [nc.dram_tensor(f"ccP{i}", [N_NODES, 256], f32,
                            kind="Internal", addr_space="Shared")
             for i in range(NITERS)]

    # tiles touching edge-tile t (inverse of windows)
    tiles_of_t = [[] for _ in range(T)]
    for n in range(NT):
        lo, hi = windows[n]
        for t in range(lo, hi):
            tiles_of_t[t].append(n)

    with tile.TileContext(nc) as tc:
        with tc.tile_pool(name="persist", bufs=1) as pp, \
             tc.tile_pool(name="work", bufs=3) as wk, \
             tc.tile_pool(name="bem_p", bufs=10) as bemp, \
             tc.tile_pool(name="sprime", bufs=6) as spp, \
             tc.tile_pool(name="stq_p", bufs=12) as spq, \
             tc.tile_pool(name="pbig", bufs=3, space="PSUM") as pbig, \
             tc.tile_pool(name="pstat", bufs=2, space="PSUM") as pstat, \
             tc.tile_pool(name="ptrans", bufs=2, space="PSUM") as ptrans, \
             tc.tile_pool(name="pscore", bufs=1, space="PSUM") as pscore:

            # ---- load constants ----
            def ld(shape, dt, dram_ap, tag):
                t_ = pp.tile(shape, dt, tag=tag)
                nc.sync.dma_start(t_[:], dram_ap)
                return t_

            c_recv = ld([P, T], f32, d_recv[:, :], "c_recv")
            c_src = ld([P, T], i32, d_src[:, :], "c_src")
            c_iota = ld([P, P], f32, d_iota[:, :], "c_iota")
            c_ndr = ld([P, NT], f32, d_ndr[:, :], "c_ndr")
            c_invdeg = ld([P, NT], f32, d_invdeg[:, :], "c_invdeg")
            c_hhT = ld([2, NPC], f16, d_hhT[:, :], "c_hhT")
            c_id16 = ld([P, P], f16, d_id16[:, :], "c_id16")
            c_id32 = ld([P, P], f32, d_id32[:, :], "c_id32")
            c_eaw1 = [ld([P, 256], f16, d_eaw1[k * P:(k + 1) * P, :], f"c_eaw1{k}") for k in range(2)]
            c_eab1 = ld([P, 2], f32, d_eab1[:, :], "c_eab1")
            c_eaw2 = pp.tile([P, 2], f16, tag="c_eaw2")
            for k in range(2):
                nc.sync.dma_start(c_eaw2[:, k:k + 1], d_eaw2[k * P:(k + 1) * P, 0:1])
            c_wa = [ld([P, 256], f16, d_wa[k * P:(k + 1) * P, :], f"c_wa{k}") for k in range(2)]
            c_wa2 = ld([2, 256], f16, d_wa2[:, :], "c_wa2")
            c_wb = [ld([P, 256], f16, d_wb[k * P:(k + 1) * P, :], f"c_wb{k}") for k in range(2)]
            c_wb2 = ld([2, 256], f16, d_wb2[:, :], "c_wb2")
            c_wc = [ld([P, 256], f16, d_wc[k * P:(k + 1) * P, :], f"c_wc{k}") for k in range(2)]
            c_eub1 = ld([P, 2], f32, d_eub1[:, :], "c_eub1")
            c_euw2 = [ld([P, 256], f16, d_euw2[k * P:(k + 1) * P, :], f"c_euw2{k}") for k in range(2)]
            c_eub2 = ld([P, 2], f32, d_eub2[:, :], "c_eub2")
            c_eww = pp.tile([P, 2], f16, tag="c_eww")
            for k in range(2):
                nc.sync.dma_start(c_eww[:, k:k + 1], d_eww[k * P:(k + 1) * P, 0:1])

            b_ea2 = pp.tile([P, 1], f32, tag="b_ea2")
            nc.vector.memset(b_ea2[:], float(_CACHE["ea_b2"]))
            b_ewb = pp.tile([P, 1], f32, tag="b_ewb")
            nc.vector.memset(b_ewb[:], float(_CACHE["ew_b"]))
            ex_all = pp.tile([P, T], f32, tag="ex_all")
            bv_all = pp.tile([P, NT * 256], f16, tag="bv_all")
            q_all = pp.tile([P, NT * 256], f16, tag="q_all")
            we_all = pp.tile([P, T], f32, tag="we_all")

            Relu = mybir.ActivationFunctionType.Relu
            Sigm = mybir.ActivationFunctionType.Sigmoid
            Expf = mybir.ActivationFunctionType.Exp
            Ident = mybir.ActivationFunctionType.Identity
            EQ = mybir.AluOpType.is_equal
            MUL = mybir.AluOpType.mult

            for it in range(NITERS):
                be_cur = be_seq[it]
                be_nxt = be_seq[(it + 1) % 3]
                last = it == NITERS - 1

                # ---------- Phase A: scores + scatter stats ----------
                stats = {}      # node-tile -> psum tile
                bem_tiles = {}  # edge-tile -> [128,257] f16 sbuf (ones|b_e)
                for g in range(G):
                    es = g * 512
                    beg = [wk.tile([P, 512], f16, tag=f"beg{k}", name=f"beg{k}") for k in range(2)]
                    for k in range(2):
                        nc.sync.dma_start(beg[k][:], be_cur[k, :, es:es + 512])
                    # h1T = relu(ea_w1.T @ b_e + b1)  (feature-major)
                    h1 = []
                    for m in range(2):
                        ph = pbig.tile([P, 512], f32, tag="pbig")
                        for k in range(2):
                            nc.tensor.matmul(ph[:], c_eaw1[k][:, m * P:(m + 1) * P],
                                             beg[k][:], start=(k == 0), stop=(k == 1))
                        hs = wk.tile([P, 512], f16, tag=f"h1s{m}")
                        nc.scalar.activation(hs[:], ph[:], Relu,
                                             bias=c_eab1[:, m:m + 1])
                        h1.append(hs)
                    # score head -> psum [128,4]; sigmoid; exp
                    psc = pscore.tile([P, 4], f32, tag="pscore")
                    for tl in range(4):
                        for k in range(2):
                            nc.tensor.matmul(psc[:, tl:tl + 1],
                                             h1[k][:, tl * P:(tl + 1) * P],
                                             c_eaw2[:, k:k + 1],
                                             start=(k == 0), stop=(k == 1))
                    ssig = wk.tile([P, 4], f32, tag="ssig")
                    nc.scalar.activation(ssig[:], psc[:], Sigm, bias=b_ea2[:, 0:1])
                    nc.scalar.activation(ex_all[:, es // P: es // P + 4], ssig[:], Expf)

                    # transpose b_e tiles to edge-major [e, 257] (ones | b_e)
                    for tl in range(4):
                        t = g * 4 + tl
                        pT = ptrans.tile([P, 256], f16, tag="ptrans")
                        for k in range(2):
                            nc.tensor.transpose(pT[:, k * P:(k + 1) * P],
                                                beg[k][:, tl * P:(tl + 1) * P],
                                                c_id16[:])
                        bem = bemp.tile([P, 257], f16, tag="bem")
                        nc.vector.memset(bem[:, 0:1], 1.0)
                        nc.vector.tensor_copy(bem[:, 1:257], pT[:])
                        bem_tiles[t] = bem
                        # scatter into all node tiles this edge tile touches
                        for n in tiles_of_t[t]:
                            if n not in stats:
                                stats[n] = pstat.tile([P, 257], f32, tag="pstat", name=f"stat{n}")
                            shift = spp.tile([P, 1], f32, tag="shift")
                            nc.vector.tensor_scalar_add(shift[:], c_recv[:, t:t + 1],
                                                        float(-128.0 * n))
                            sp = spp.tile([P, P], f16, tag="sp")
                            nc.vector.tensor_scalar(
                                out=sp[:], in0=c_iota[:], scalar1=shift[:, 0:1],
                                scalar2=ex_all[:, t:t + 1], op0=EQ, op1=MUL)
                            first = t == windows[n][0]
                            lastp = t == last_tile[n]
                            nc.tensor.matmul(stats[n][:], sp[:], bem[:],
                                             start=first, stop=lastp,
                                             skip_group_check=True)
                            if lastp:
                                # finalize node tile: b_v = stats[:,1:]/(clamp(s0)*(1+deg))
                                st = stats.pop(n)
                                sc = spp.tile([P, 1], f32, tag="sc")
                                nc.vector.tensor_scalar_max(sc[:], st[:, 0:1], 1e-6)
                                rec = spp.tile([P, 1], f32, tag="rec")
                                nc.vector.reciprocal(rec[:], sc[:])
                                scl = spp.tile([P, 1], f32, tag="scl")
                                nc.vector.tensor_tensor(
                                    out=scl[:], in0=rec[:], in1=c_invdeg[:, n:n + 1], op=MUL)
                                nc.vector.tensor_scalar(
                                    out=bv_all[:, n * 256:(n + 1) * 256],
                                    in0=st[:, 1:257], scalar1=scl[:, 0:1],
                                    scalar2=None, op0=MUL)
                                if last:
                                    bvf = wk.tile([P, 256], f32, tag="bvf")
                                    nc.vector.tensor_scalar(
                                        out=bvf[:], in0=st[:, 1:257],
                                        scalar1=scl[:, 0:1], scalar2=None, op0=MUL)
                                    nc.sync.dma_start(
                                        d_bv[n * P:(n + 1) * P, :], bvf[:])

                # ---------- Phase B: P,Q node tables + AllGather ----------
                for n in range(NT):
                    pT = ptrans.tile([P, 256], f16, tag="ptrans")
                    for k in range(2):
                        nc.tensor.transpose(pT[:, k * P:(k + 1) * P],
                                            bv_all[:, n * 256 + k * P: n * 256 + (k + 1) * P],
                                            c_id16[:])
                    bvT = wk.tile([P, 256], f16, tag="bvT")
                    nc.vector.tensor_copy(bvT[:], pT[:])
                    for (dst_all, wlist, w2) in ((None, c_wa, c_wa2), (q_all, c_wb, c_wb2)):
                        pq = pbig.tile([P, 256], f32, tag="pbig")
                        nc.tensor.matmul(pq[:], bvT[:, 0:P], wlist[0][:], start=True, stop=False)
                        nc.tensor.matmul(pq[:], bvT[:, P:256], wlist[1][:], start=False, stop=False)
                        nc.tensor.matmul(pq[:], c_hhT[:, n * P:(n + 1) * P], w2[:],
                                         start=False, stop=True)
                        if dst_all is None:
                            pf = wk.tile([P, 256], f32, tag="pf")
                            nc.vector.tensor_copy(pf[:], pq[:])
                            nc.sync.dma_start(d_ccin[it][n * P:(n + 1) * P, :], pf[:])
                        else:
                            nc.vector.tensor_copy(dst_all[:, n * 256:(n + 1) * 256], pq[:])
                nc.gpsimd.collective_compute(
                    "AllGather", mybir.AluOpType.bypass,
                    ins=[d_ccin[it][:]], outs=[d_ccP[it][:]],
                    replica_groups=[list(range(NCORE))])

                # ---------- Phase C: edge update MLP ----------
                for g in range(G):
                    es = g * 512
                    beg = [wk.tile([P, 512], f16, tag=f"beg{k}", name=f"beg{k}") for k in range(2)]
                    for k in range(2):
                        nc.sync.dma_start(beg[k][:], be_cur[k, :, es:es + 512])
                    pg = []
                    for tl in range(4):
                        t = g * 4 + tl
                        pgt = wk.tile([P, 256], f32, tag=f"pg{tl}")
                        nc.gpsimd.indirect_dma_start(
                            out=pgt[:], out_offset=None, in_=d_ccP[it][:],
                            in_offset=bass.IndirectOffsetOnAxis(
                                ap=c_src[:, t:t + 1], axis=0))
                        pg.append(pgt)
                    # S^T tiles per (t, n) — shared across both fc chunks
                    st_tiles = {}
                    for tl in range(4):
                        t = g * 4 + tl
                        prr = ptrans.tile([P, P], f32, tag="ptrans")
                        nc.tensor.transpose(
                            prr[:], c_recv[:, t:t + 1].to_broadcast([P, P]), c_id32[:])
                        for n in tiles_of_t[t]:
                            st_ = spq.tile([P, P], f16, tag="stq")
                            nc.vector.tensor_scalar(
                                out=st_[:], in0=prr[:],
                                scalar1=c_ndr[:, n:n + 1], scalar2=None, op0=EQ)
                            st_tiles[(tl, n)] = st_
                    hpre = []
                    for fc in range(2):
                        ph = pbig.tile([P, 512], f32, tag="pbig")
                        # W_c^T b_e
                        for k in range(2):
                            nc.tensor.matmul(ph[:], c_wc[k][:, fc * P:(fc + 1) * P],
                                             beg[k][:], start=(k == 0), stop=False,
                                             skip_group_check=True)
                        # Q expand + P gather add
                        for tl in range(4):
                            t = g * 4 + tl
                            for n in tiles_of_t[t]:
                                nc.tensor.matmul(
                                    ph[:, tl * P:(tl + 1) * P],
                                    q_all[:, n * 256 + fc * P: n * 256 + (fc + 1) * P],
                                    st_tiles[(tl, n)][:], start=False, stop=False,
                                    skip_group_check=True)
                            nc.tensor.matmul(
                                ph[:, tl * P:(tl + 1) * P],
                                pg[tl][:, fc * P:(fc + 1) * P], c_id32[:],
                                is_transpose=True, start=False,
                                stop=(tl == 3), skip_group_check=True)
                        hs = wk.tile([P, 512], f16, tag=f"hpre{fc}")
                        nc.scalar.activation(hs[:], ph[:], Relu,
                                             bias=c_eub1[:, fc:fc + 1])
                        hpre.append(hs)
                    # b_e' = euw2.T @ h + b2
                    ben = []
                    for fc in range(2):
                        pb = pbig.tile([P, 512], f32, tag="pbig")
                        for k in range(2):
                            nc.tensor.matmul(pb[:], c_euw2[k][:, fc * P:(fc + 1) * P],
                                             hpre[k][:], start=(k == 0), stop=(k == 1))
                        bs = wk.tile([P, 512], f16, tag=f"ben{fc}")
                        nc.scalar.activation(bs[:], pb[:], Ident,
                                             bias=c_eub2[:, fc:fc + 1])
                        ben.append(bs)
                        if not last:
                            nc.sync.dma_start(be_nxt[fc, :, es:es + 512], bs[:])
                    if last:
                        # w_e = sigmoid(b_e' @ ew_w + ew_b)
                        pw = pscore.tile([P, 4], f32, tag="pscore")
                        for tl in range(4):
                            for k in range(2):
                                nc.tensor.matmul(pw[:, tl:tl + 1],
                                                 ben[k][:, tl * P:(tl + 1) * P],
                                                 c_eww[:, k:k + 1],
                                                 start=(k == 0), stop=(k == 1))
                        nc.scalar.activation(we_all[:, es // P: es // P + 4],
                                             pw[:], Sigm, bias=b_ewb[:, 0:1])
            nc.sync.dma_start(d_we[:, :], we_all[:])

    _split_multi_waits(nc, mybir)
    return nc


def kernel(**inputs):
    import sys
    if '/opt/trn_rl_repo' not in sys.path:
        sys.path.insert(0, '/opt/trn_rl_repo')
    from concourse.bass_utils import run_bass_kernel_spmd
    nc, in_maps, perm, counts, starts = _prepare(inputs)
    res = run_bass_kernel_spmd(nc, in_maps, core_ids=list(range(NCORE)))
    _CACHE["exec_time_ns"] = res.exec_time_ns
    return _postprocess(res.results, inputs, perm, counts, starts)


def _prepare(inputs):
    edge_features = np.asarray(inputs["edge_features"], np.float32)
    g_all = np.asarray(inputs["g_all"], np.float32)
    edge_index = np.asarray(inputs["edge_index"], np.int32)
    node_degrees = np.asarray(inputs["node_degrees"], np.int32)
    batch = np.asarray(inputs["batch"], np.int32)
    heads = np.asarray(inputs["heads"], np.int32)
    tails = np.asarray(inputs["tails"], np.int32)

    src, recv = edge_index[0], edge_index[1]
    E = src.shape[0]
    perm = np.argsort(recv, kind="stable")
    recv_s = recv[perm].astype(np.int64)
    src_s = src[perm].astype(np.int64)
    core_of = recv_s // NPC
    counts = np.bincount(core_of, minlength=NCORE)
    starts = np.concatenate([[0], np.cumsum(counts)])
    EC = int(np.ceil(counts.max() / 512) * 512)
    T = EC // P

    # per-core packed inputs
    b_e0 = np.concatenate(
        [edge_features, np.broadcast_to(g_all, (E, g_all.shape[1]))], axis=1)

    # edge-tile windows per node tile, uniform across cores
    win_lo = np.full(NT, 10 ** 9, np.int64)
    win_hi = np.zeros(NT, np.int64)
    per_core = []
    for c in range(NCORE):
        sl = slice(starts[c], starts[c + 1])
        rl = recv_s[sl] - c * NPC
        nt_of_edge = rl // P
        e_local = np.arange(counts[c])
        for n in range(NT):
            m = nt_of_edge == n
            if m.any():
                el = e_local[m]
                win_lo[n] = min(win_lo[n], el[0] // P)
                win_hi[n] = max(win_hi[n], el[-1] // P + 1)
        per_core.append((sl, rl))
    win_lo = np.minimum(win_lo, T - 1)
    win_hi = np.maximum(win_hi, win_lo + 1)
    windows = [(int(win_lo[n]), int(win_hi[n])) for n in range(NT)]
    last_tile = [int(win_hi[n] - 1) for n in range(NT)]
    for n in range(NT - 2):
        # pstat pool has 2 buffers: windows two apart must not overlap
        assert windows[n + 2][0] > last_tile[n], (n, windows[n], windows[n + 2])

    _CACHE["ea_b2"] = float(np.asarray(inputs["ea_b2"]).reshape(-1)[0])
    _CACHE["ew_b"] = float(np.asarray(inputs["ew_b"]).reshape(-1)[0])

    nc = _build_nc(EC, windows, last_tile)

    def colmat(vals, fill, dt):
        out = np.full(EC, fill, dt)
        out[:len(vals)] = vals
        return out.reshape(T, P).T.copy()

    ea_w1 = np.asarray(inputs["ea_w1"], np.float32)
    ea_b1 = np.asarray(inputs["ea_b1"], np.float32)
    ea_w2 = np.asarray(inputs["ea_w2"], np.float32)
    eu_w1 = np.asarray(inputs["eu_w1"], np.float32)
    eu_b1 = np.asarray(inputs["eu_b1"], np.float32)
    eu_w2 = np.asarray(inputs["eu_w2"], np.float32)
    eu_b2 = np.asarray(inputs["eu_b2"], np.float32)
    ew_w = np.asarray(inputs["ew_w"], np.float32)

    iota = np.broadcast_to(np.arange(P, dtype=np.float32)[None, :], (P, P)).copy()
    ndr = (np.arange(NPC, dtype=np.float32).reshape(NT, P).T).copy()
    id16 = np.eye(P, dtype=np.float16)
    id32 = np.eye(P, dtype=np.float32)

    # head/tail indicator per node
    node_ids = np.arange(N_NODES, dtype=np.int64)
    is_head = (node_ids == heads[batch]).astype(np.float16)
    is_tail = (node_ids == tails[batch]).astype(np.float16)

    shared = dict(
        iota=iota, ndr=ndr, id16=id16, id32=id32,
        eaw1=ea_w1.astype(np.float16), eab1=ea_b1.reshape(2, P).T.copy(),
        eaw2=ea_w2.astype(np.float16),
        wa=eu_w1[0:256].astype(np.float16), wa2=eu_w1[256:258].astype(np.float16),
        wb=eu_w1[258:514].astype(np.float16), wb2=eu_w1[514:516].astype(np.float16),
        wc=eu_w1[516:772].astype(np.float16),
        eub1=eu_b1.reshape(2, P).T.copy(),
        euw2=eu_w2.astype(np.float16),
        eub2=eu_b2.reshape(2, P).T.copy(),
        eww=ew_w.astype(np.float16),
    )

    in_maps = []
    for c in range(NCORE):
        sl, rl = per_core[c]
        nct = counts[c]
        be_c = np.zeros((EC, 256), np.float32)
        be_c[:nct] = b_e0[perm[sl]]
        beT = be_c.T.reshape(2, P, EC).astype(np.float16)
        recvc = colmat(rl.astype(np.float32), -100000.0, np.float32)
        srcc = colmat(src_s[sl].astype(np.int32), 0, np.int32)
        invdeg = (1.0 / (1.0 + node_degrees[c * NPC:(c + 1) * NPC].astype(np.float32))
                  ).reshape(NT, P).T.copy()
        hhT = np.stack([is_head[c * NPC:(c + 1) * NPC],
                        is_tail[c * NPC:(c + 1) * NPC]], axis=0)
        m = dict(be0=beT, recvc=recvc, srcc=srcc, invdeg=invdeg, hhT=hhT)
        m.update(shared)
        in_maps.append(m)

    return nc, in_maps, perm, counts, starts


def _postprocess(results, inputs, perm, counts, starts):
    E = N_EDGES
    batch = np.asarray(inputs["batch"], np.int64)
    heads = np.asarray(inputs["heads"], np.int64)
    tails = np.asarray(inputs["tails"], np.int64)
    w_e_sorted = np.empty(E, np.float32)
    b_v = np.empty((N_NODES, 256), np.float32)
    for c in range(NCORE):
        r = results[c]
        wec = r["we_out"].T.reshape(-1)[:counts[c]]
        w_e_sorted[starts[c]:starts[c + 1]] = wec
        b_v[c * NPC:(c + 1) * NPC] = r["bv_out"]
    w_e = np.empty(E, np.float32)
    w_e[perm] = w_e_sorted
    w_e = w_e[:, None]

    bv64 = b_v.astype(np.float64)
    expected_batch = np.repeat(np.arange(N_GRAPHS), N_NODES // N_GRAPHS)
    if np.array_equal(batch, expected_batch):
        g_max = bv64.reshape(N_GRAPHS, N_NODES // N_GRAPHS, 256).max(axis=1)
    else:
        g_max = np.full((N_GRAPHS, 256), -np.inf)
        np.maximum.at(g_max, batch, bv64)
    g_G = np.concatenate([g_max, bv64[heads], bv64[tails]], axis=1)  # [G,768]
    gw_w1 = np.asarray(inputs["gw_w1"], np.float64)
    gw_b1 = np.asarray(inputs["gw_b1"], np.float64)
    gw_w2 = np.asarray(inputs["gw_w2"], np.float64)
    gw_b2 = np.asarray(inputs["gw_b2"], np.float64)
    z = np.maximum(g_G @ gw_w1 + gw_b1, 0.0) @ gw_w2 + gw_b2  # [G,1]
    zm = z - z.max(axis=0, keepdims=True)
    wk = np.exp(zm)
    wk = wk / wk.sum(axis=0, keepdims=True)
    g_all_out = (wk * g_G).sum(axis=0, keepdims=True)

    return (w_e.astype(np.float32), g_all_out.astype(np.float32))
